# revision 35
# baseline (speedup 1.0000x reference)
"""BFP-quantized linear layer (BFLinear) for Trainium2, 8-core data-parallel.

Computes: out = bfp_q(x, 8, 16) @ bfp_q(w, 8, 16).T + bias
  where bfp_q groups 16 contiguous elements along the feature axis, shares
  exponent e = floor(log2(max|g|)), rounds mantissas to `bit` bits (RNE) and
  clips to [-2^(bit-1), 2^(bit-1)-1].

Division of labor:
  - weights (small constant parameters) are BFP-quantized on the host with
    the exact reference formula and shipped pre-transposed as bf16 (wq values
    are int*2^e, bf16-exact);
  - bias is added on the host during the bf16->f32 output upcast (exact);
  - everything else (quantization of x, matmul) runs on-device.

On-device math per 512-row chunk (tile [128, 2048], rows on partitions):
  gmax  = max|group|               DVE reduce, f32 (exact exponents)
  rb0   = bits(gmax)&EM ^ EM       DVE TS = f32 2^(1-e)    } one [P,2,G] tile,
  pe    = rb0 ^ EM                 DVE TS = f32 2^e        } one dup-cast to
                                   fp16 "pairs" (each value 2x -> the TT reads
                                   coalesced 4B pairs at 2 elem/cycle)
  xb    = fp16(x)                  ACT copy (deep-skewed off the chain)
  v     = xb * 2^(1-e)             DVE TT pairs, fp16 (exact pow2 scaling of
                                   fp16; 1215ns vs 2291ns reading f32)
  u     = clamp(v, -2, 127/64)     Pool TS min/max (1-stream op coexists
                                   with DVE; bounds fp16-exact)
  t     = (u + C') + (-C') -> fp16 DVE TS add/add, C'=1.5*2^17 forces RNE on
                                   the 2^-(bit-2) grid
  xq    = t * 2^e -> bf16          DVE TT pairs (= 2x reference xq, bf16-exact)
  xqT   = PE transpose -> PSUM, ACT copy -> SBUF
  out   = xq @ wqT via 4x4 accumulating PE matmuls into PSUM f32
  outsb = ACT copy PSUM -> bf16 with scale=0.5 (fixes the 2x, exact pow2)
Output written bf16 (halves HBM write traffic), upcast + bias on host.

fp16 intermediates: RNE_int(fp16(x)*2^k) double-rounds vs the reference's
RNE_int(x*2^k); with fp16's 11-bit mantissa this perturbs ~0.5% of elements
by one grid step (max abs err 0.021 vs 0.016 for the exact-f32 path; rel err
3.4e-3 max-scaled / 1.2e-2 with denominator floored at 1). Set CFG["vdt"] to
"f32" for the bit-matched (slower) path.

Clamp-before-round with bounds (-128, 127)*2^-(bit-2) equals the reference's
round-then-clip for every case incl. ties.

Schedule: items of 4 row-blocks (halves at both ends to shorten pipeline fill
and drain), DMA 4 items ahead, quant_a (reduce/smalls/pairs/convert) 2 items
ahead of quant_b (mult/clamp/round/scale/transpose), matmuls one item behind.
Steady state is DVE-bound at ~5.7us/chunk with ACT ~equal (convert + xqT copy
+ outcopy); measured span ~126us vs ~190us for the previous baseline.

Hardware facts learned from traces (violating any costs 2-25x):
  - DVE and GpSimd share one SBUF port: concurrent heavy ops degrade toward
    the combined serial rate. Keep GpSimd to at most one light 1-stream TS
    (the clamp); its TT/CAST ops are slow (4.6us/7.3us per [128,2048]) and
    poison DVE. All other elementwise work belongs on DVE at clean rates.
  - DVE TT with a [0,16]-broadcast operand runs 1 elem/cycle regardless of
    dtype; duplicating the broadcast values ("pairs", inner AP [0,8][1,2])
    restores 2 elem/cycle for 16-bit. TS (1-stream) ops hit ~3 elem/cycle
    16-bit, ~1.7 f32. f32 streams cap at 1 elem/cycle. f32-out TTs from
    16-bit inputs run half rate. tensor_reduce is 1 elem/cycle regardless.
  - int8 is no faster: CAST->int8 1228ns, TT int8 in0 2302ns ([128,2048]),
    though CAST bf16->int8 is exactly clip(RNE(v),-128,127) if ever useful.
  - PE matmuls pipeline to ~215ns per [128c,512f] bf16 when rhs is a
    contiguous tile; strided rhs APs halve throughput. DoubleRow perf mode
    (2x) is fp8-only on TRN2.
  - dma_start_transpose (xbar, 2-byte dtypes, out[do,di,m]=in[m,di*128+do])
    works but costs ~2x normal DMA engine-time per byte; transposing every
    chunk saturated all 16 DMA engines (~127us each). PE transpose + ACT
    PSUM->SBUF copy is cheaper when ACT has headroom.
  - DMA cannot read PSUM (API assert); GpSimd TS from PSUM fails NEFF
    codegen; ACT reads PSUM at ~1 elem/cycle. Matmul operands must be SBUF.
  - TS cannot mix bitwise and arithmetic ops in one instruction; TS on Pool
    only does min/max; no negative int immediates on TS; scalar slots of
    activation() take [P,1] APs or immediates - exact-pow2 scale immediates
    ride ACT copies for free (used for the 0.5 output fix-up).
  - Upfront DMA bursts round-robin packets across engines: a tensor needed
    first should be dispatched first (x0 before the other prefetches).
"""

import os
import sys

import numpy as np

for _p in ("/opt/trn_rl_repo",):
    if _p not in sys.path and os.path.isdir(_p):
        sys.path.append(_p)

N_CORES = 8

# DVE and GpSimd share one SBUF port (concurrent ops degrade to the
# combined serial rate), so all elementwise work runs on DVE at clean rates
# and GpSimd stays idle. ACT (own port) does convert/pairs/outcopy.
CFG = {
    "vdt": "bf16",     # v/u/t dtype: bf16 (fast, double-round) | f32 (exact)
    "mult": "v",
    "clamp": "g",
    "round": "v",
    "scale_dve_groups": 128,  # of 128 groups/chunk: DVE share of scale TT
    "outcopy": "a",
}

_CACHE = {}


def _eng(nc, which, idx=0):
    s = {"v": nc.vector, "g": nc.gpsimd, "a": nc.scalar}
    return s[which[idx % len(which)]]


def _bcast_group_ap(t, G, sz):
    """AP reading tile t[P, G] as [P, G, sz] with the last dim broadcast."""
    import concourse.bass as bass

    ap = t.ap.copy()
    ap.append([0, sz])
    return bass.AP(tensor=t.tensor, offset=t.offset, ap=ap)


def _pairs_ap(t, g0, ng):
    """Read pairs-packed tile t[P, 2G] (each group value duplicated) as
    [P, ng, 8, 2] covering groups [g0, g0+ng): coalesced 4B pair reads."""
    import concourse.bass as bass

    view = t[:, 2 * g0 : 2 * (g0 + ng)]
    ap = view.ap.copy()[:1]
    ap.append([2, ng])
    ap.append([0, 8])
    ap.append([1, 2])
    return bass.AP(tensor=view.tensor, offset=view.offset, ap=ap)


def _build(nrows, K, O, x_bit, w_bit, x_sz, w_sz, cfg=None):
    import concourse.bacc as bacc
    import concourse.bass as bass  # noqa: F401
    import concourse.mybir as mybir
    import concourse.tile as tile
    from concourse.masks import make_identity

    cfg = dict(CFG, **(cfg or {}))
    f32 = mybir.dt.float32
    bf16 = mybir.dt.bfloat16
    i32 = mybir.dt.int32
    A = mybir.AluOpType

    assert x_bit == w_bit == 8 and x_sz == w_sz == 16
    bit, sz = x_bit, x_sz
    EM = 0x7F800000
    KADD = (bit - 2) << 23
    # v = x*2^(1-e) lives on the 2^-(bit-2) grid: C and the clamp bounds are
    # scaled by 2^-(bit-2); xq = t*2^e comes out 2x the reference, fixed by a
    # 0.5 scale immediate in the outcopy (all exact powers of two).
    C = float(np.float32(1.5 * 2.0 ** (23 - (bit - 2))))
    qhi = float((2 ** (bit - 1) - 1) * 2.0 ** (-(bit - 2)))
    qlo = float(-(2 ** (bit - 1)) * 2.0 ** (-(bit - 2)))

    P = 128
    RPC = 512
    assert nrows % RPC == 0
    n_chunks = nrows // RPC
    FB = RPC // P            # 4 row-blocks per chunk
    F = FB * K               # 2048 free columns per chunk
    G = F // sz              # 128 groups per chunk
    KB = K // P              # 4 k-blocks
    OB = O // P              # 4 o-blocks
    GW = K // sz             # 32 groups per weight row-tile
    f16 = mybir.dt.float16
    # fp16 intermediates: 11-bit mantissa cuts the pre-round double-rounding
    # error ~8x vs bf16 (maxerr 0.021 vs 0.035) at identical 16-bit DVE rates
    vdt = f16 if cfg["vdt"] == "bf16" else f32
    sdg = cfg["scale_dve_groups"]

    nc = bacc.Bacc("TRN2", debug=False)
    x_d = nc.dram_tensor("x", (nrows, K), f32, kind="ExternalInput").ap()
    wt_d = nc.dram_tensor("wqt", (K, O), bf16, kind="ExternalInput").ap()
    o_d = nc.dram_tensor("out", (nrows, O), bf16, kind="ExternalOutput").ap()

    with tile.TileContext(nc) as tc:
        with (
            tc.tile_pool(name="const", bufs=1) as constp,
            tc.tile_pool(name="xraw", bufs=4) as xraw,
            tc.tile_pool(name="sml", bufs=4) as sml,
            tc.tile_pool(name="xb", bufs=4) as xbp,
            tc.tile_pool(name="prs", bufs=4) as prs,
            tc.tile_pool(name="v", bufs=3) as vp,
            tc.tile_pool(name="u", bufs=2) as up,
            tc.tile_pool(name="t", bufs=2) as tp,
            tc.tile_pool(name="xq", bufs=4) as xqp,
            tc.tile_pool(name="xqT", bufs=4) as xqTp,
            tc.tile_pool(name="osb", bufs=4) as osb,
            tc.tile_pool(name="psO", bufs=2, space="PSUM") as psO,
            tc.tile_pool(name="psT", bufs=2, space="PSUM") as psT,
        ):
            p2s = constp.tile([P, 1], f32)
            nc.vector.memset(p2s, float(2.0 ** (-(bit - 1))))
            r2s = constp.tile([P, 1], f32)
            nc.vector.memset(r2s, float(2.0 ** (bit - 2)))
            ident = constp.tile([P, P], bf16)
            make_identity(nc, ident)

            # ---- software-pipelined main loop over items (r0, nf) ----
            # First and last chunks are split into halves: a shorter first
            # chain fills the pipeline sooner, a shorter last chain halves
            # the serial drain tail.
            items = (
                [(0, 1), (P, 1), (2 * P, 1), (3 * P, 1)]
                + [(c * RPC, FB) for c in range(1, n_chunks - 1)]
                + [((n_chunks - 1) * RPC, FB // 2),
                   ((n_chunks - 1) * RPC + RPC // 2, FB // 2)]
            )
            n_items = len(items)
            st = {}

            def dma_in(i):
                r0, nf = items[i]
                x_raw = xraw.tile([P, FB, K], f32, tag="x_raw", name="x_raw")[:, :nf]
                src = x_d[r0 : r0 + nf * P, :].rearrange("(f p) k -> p f k", p=P)
                nc.sync.dma_start(out=x_raw, in_=src)
                st[i] = {"x": x_raw}

            def quant_a(i):
                r0, nf = items[i]
                Fi = nf * K
                Gi = Fi // sz
                s = st[i]
                xt = s["x"].rearrange("p f k -> p (f k)")
                s["xt"] = xt
                gmx = sml.tile([P, F // sz], f32, tag="gmx", name="gmx")[:, :Gi]
                nc.vector.tensor_reduce(
                    out=gmx,
                    in_=xt.rearrange("p (g s) -> p g s", s=sz),
                    axis=mybir.AxisListType.X,
                    op=A.max,
                    apply_absolute_value=True,
                )
                sc2 = sml.tile([P, 2, Gi], i32, tag=f"sc2{nf}", name="sc2")
                nc.vector.tensor_scalar(
                    out=sc2[:, 0], in0=gmx.bitcast(i32), scalar1=EM, scalar2=EM,
                    op0=A.bitwise_and, op1=A.bitwise_xor,
                )
                nc.vector.tensor_scalar(
                    out=sc2[:, 1], in0=sc2[:, 0], scalar1=EM, scalar2=None,
                    op0=A.bitwise_xor,
                )
                # x -> bf16 on ACT (deep-skewed off the critical chain);
                # the mult then runs 1460ns (bf16 pairs) vs 2291ns (f32 in0)
                xb = xbp.tile([P, F], f16, tag="xb", name="xb")[:, :Fi]
                nc.scalar.copy(xb, xt)
                s["xb"] = xb
                prt = prs.tile([P, 2, 2 * Gi], f16, tag=f"prt{nf}", name="prt")
                nc.vector.tensor_copy(
                    out=prt.rearrange("p a b -> p (a b)"),
                    in_=_bcast_group_ap(
                        sc2.rearrange("p a b -> p (a b)").bitcast(f32), 2 * Gi, 2
                    ),
                )
                s["rbp"], s["scp"] = prt[:, 0], prt[:, 1]

            def quant_b(i):
                r0, nf = items[i]
                Fi = nf * K
                Gi = Fi // sz
                s = st.pop(i)
                v = vp.tile([P, F], vdt, tag="v", name="v")[:, :Fi]
                _eng(nc, cfg["mult"], i).tensor_tensor(
                    out=v, in0=s["xb"], in1=_pairs_ap(s["rbp"], 0, Gi), op=A.mult,
                )
                u = up.tile([P, F], vdt, tag="u", name="u")[:, :Fi]
                _eng(nc, cfg["clamp"], i).tensor_scalar(
                    out=u, in0=v, scalar1=qhi + 0.0, scalar2=qlo + 0.0,
                    op0=A.min, op1=A.max,
                )
                t = tp.tile([P, F], f16, tag="t", name="t")[:, :Fi]
                _eng(nc, cfg["round"], i).tensor_scalar(
                    out=t, in0=u, scalar1=C, scalar2=-C, op0=A.add, op1=A.add,
                )
                xq = xqp.tile([P, F], bf16, tag="xq", name="xq")[:, :Fi]
                sd = (sdg * nf) // FB
                if sd > 0:
                    nc.vector.tensor_tensor(
                        out=xq[:, : sd * sz], in0=t[:, : sd * sz],
                        in1=_pairs_ap(s["scp"], 0, sd), op=A.mult,
                    )
                if sd < Gi:
                    nc.gpsimd.tensor_tensor(
                        out=xq[:, sd * sz :], in0=t[:, sd * sz :],
                        in1=_pairs_ap(s["scp"], sd, Gi - sd), op=A.mult,
                    )
                xq_nat = xq.rearrange("p (f c q) -> p f c q", f=nf, c=KB)
                ptT_full = psT.tile([P, FB * KB, P], bf16, tag="ptT")
                ptT = ptT_full[:, : nf * KB, :]
                for fb in range(nf):
                    for kb in range(KB):
                        nc.tensor.transpose(
                            ptT[:, fb * KB + kb, :], xq_nat[:, fb, kb], ident
                        )
                xqT = xqTp.tile([P, FB * KB, P], bf16, tag="xqT", name="xqT")[:, : nf * KB]
                nc.scalar.copy(xqT, ptT)
                st[i] = {"xqT": xqT}

            def mm_out(i):
                r0, nf = items[i]
                s = st.pop(i)
                xqT = s["xqT"]
                for fp in range(nf // 2):
                    po = psO.tile([P, 2, O], f32, tag="po")
                    for g in range(2):
                        fb = fp * 2 + g
                        for kb in range(KB):
                            nc.tensor.matmul(
                                po[:, g, :],
                                lhsT=xqT[:, fb * KB + kb, :],
                                rhs=wqT[kb],
                                start=(kb == 0),
                                stop=(kb == KB - 1),
                            )
                    out_sb = osb.tile([P, 2, O], bf16, tag="out_sb")
                    nc.scalar.activation(
                        out_sb, po, mybir.ActivationFunctionType.Copy, scale=0.5
                    )
                    rr = r0 + fp * 2 * P
                    dst = o_d[rr : rr + 2 * P, :].rearrange("(f p) k -> p f k", p=P)
                    nc.sync.dma_start(out=dst, in_=out_sb)

            dma_in(0)
            # weights after x0: x0's input is on the critical path, the wqT
            # tiles aren't needed until the first matmul (~25us in)
            wqT = []
            for kb in range(KB):
                wt = constp.tile([P, O], bf16, tag=f"wqT{kb}", bufs=KB)
                nc.sync.dma_start(out=wt, in_=wt_d[kb * P : (kb + 1) * P, :])
                wqT.append(wt)
            for j in range(1, min(4, n_items)):
                dma_in(j)
            quant_a(0)
            if n_items > 1:
                quant_a(1)
            mm_done = set()

            def mm(j):
                if j not in mm_done:
                    mm_done.add(j)
                    mm_out(j)

            for i in range(n_items):
                if i + 4 < n_items:
                    dma_in(i + 4)
                quant_b(i)
                if i + 2 < n_items:
                    quant_a(i + 2)
                if i >= 1:
                    mm(i - 1)
                if i >= n_items - 2:
                    mm(i)
    nc.compile()
    return nc


def _get_program(nrows, K, O, x_bit, w_bit, x_sz, w_sz):
    key = (nrows, K, O, x_bit, w_bit, x_sz, w_sz)
    if key not in _CACHE:
        _CACHE[key] = _build(nrows, K, O, x_bit, w_bit, x_sz, w_sz)
    return _CACHE[key]


def _host_bfp_quantize(w, bit, sz):
    """Reference bfp_quantize in float32 numpy (np.round is RNE like jnp)."""
    g = w.reshape(-1, sz)
    maxabs = np.max(np.abs(g), axis=1, keepdims=True)
    e = np.floor(np.log2(np.maximum(maxabs, np.float32(1e-38), dtype=np.float32)))
    scale = np.exp2(e - (bit - 1), dtype=np.float32)
    qmax = np.float32(2.0 ** (bit - 1) - 1.0)
    q = np.clip(np.round(g / scale), -qmax - 1.0, qmax) * scale
    return q.reshape(w.shape).astype(np.float32)


def kernel(input, weight, bias, i_bit, i_sz, w_bit, w_sz):
    import ml_dtypes
    from concourse.bass_utils import run_bass_kernel_spmd

    x = np.ascontiguousarray(np.asarray(input, dtype=np.float32))
    w = np.ascontiguousarray(np.asarray(weight, dtype=np.float32))
    b = np.asarray(bias, dtype=np.float32).reshape(1, -1)
    i_bit, i_sz, w_bit, w_sz = int(i_bit), int(i_sz), int(w_bit), int(w_sz)

    N, K = x.shape
    O = w.shape[0]
    assert N % N_CORES == 0
    shard = N // N_CORES

    # wq values are int*2^e, exactly representable in bf16
    wqt = np.ascontiguousarray(
        _host_bfp_quantize(w, w_bit, w_sz).T
    ).astype(ml_dtypes.bfloat16)

    nc = _get_program(shard, K, O, i_bit, w_bit, i_sz, w_sz)
    in_maps = [
        {"x": x[i * shard : (i + 1) * shard], "wqt": wqt} for i in range(N_CORES)
    ]
    res = run_bass_kernel_spmd(nc, in_maps, list(range(N_CORES)))
    out = np.empty((N, O), dtype=np.float32)
    for i, r in enumerate(res.results):
        np.add(
            np.asarray(r["out"]).astype(np.float32), b,
            out=out[i * shard : (i + 1) * shard],
        )
    return out


# revision 36
# speedup vs baseline: 1.0002x; 1.0002x over previous
"""BFP-quantized linear layer (BFLinear) for Trainium2, 8-core data-parallel.

Computes: out = bfp_q(x, 8, 16) @ bfp_q(w, 8, 16).T + bias
  where bfp_q groups 16 contiguous elements along the feature axis, shares
  exponent e = floor(log2(max|g|)), rounds mantissas to `bit` bits (RNE) and
  clips to [-2^(bit-1), 2^(bit-1)-1].

Division of labor:
  - weights (small constant parameters) are BFP-quantized on the host with
    the exact reference formula and shipped pre-transposed as bf16 (wq values
    are int*2^e, bf16-exact);
  - bias is added on the host during the bf16->f32 output upcast (exact);
  - everything else (quantization of x, matmul) runs on-device.

On-device math per 512-row chunk (tile [128, 2048], rows on partitions):
  gmax  = max|group|               DVE reduce, f32 (exact exponents)
  rb0   = bits(gmax)&EM ^ EM       DVE TS = f32 2^(1-e)    } one [P,2,G] tile,
  pe    = rb0 ^ EM                 DVE TS = f32 2^e        } one dup-cast to
                                   fp16 "pairs" (each value 2x -> the TT reads
                                   coalesced 4B pairs at 2 elem/cycle)
  xb    = fp16(x)                  ACT copy (deep-skewed off the chain)
  v     = xb * 2^(1-e)             DVE TT pairs, fp16 (exact pow2 scaling of
                                   fp16; 1215ns vs 2291ns reading f32)
  u     = clamp(v, -2, 127/64)     Pool TS min/max (1-stream op coexists
                                   with DVE; bounds fp16-exact)
  t     = (u + C') + (-C') -> fp16 DVE TS add/add, C'=1.5*2^17 forces RNE on
                                   the 2^-(bit-2) grid
  xq    = t * 2^e -> bf16          DVE TT pairs (= 2x reference xq, bf16-exact)
  xqT   = PE transpose -> PSUM, ACT copy -> SBUF
  out   = xq @ wqT via 4x4 accumulating PE matmuls into PSUM f32
  outsb = ACT copy PSUM -> bf16 with scale=0.5 (fixes the 2x, exact pow2)
Output written bf16 (halves HBM write traffic), upcast + bias on host.

fp16 intermediates: RNE_int(fp16(x)*2^k) double-rounds vs the reference's
RNE_int(x*2^k); with fp16's 11-bit mantissa this perturbs ~0.5% of elements
by one grid step (max abs err 0.021 vs 0.016 for the exact-f32 path; rel err
3.4e-3 max-scaled / 1.2e-2 with denominator floored at 1). Set CFG["vdt"] to
"f32" for the bit-matched (slower) path.

Clamp-before-round with bounds (-128, 127)*2^-(bit-2) equals the reference's
round-then-clip for every case incl. ties.

Schedule: items of 4 row-blocks (halves at both ends to shorten pipeline fill
and drain), DMA 4 items ahead, quant_a (reduce/smalls/pairs/convert) 2 items
ahead of quant_b (mult/clamp/round/scale/transpose), matmuls one item behind.
Steady state is DVE-bound at ~5.7us/chunk with ACT ~equal (convert + xqT copy
+ outcopy); measured span ~126us vs ~190us for the previous baseline.

Hardware facts learned from traces (violating any costs 2-25x):
  - DVE and GpSimd share one SBUF port: concurrent heavy ops degrade toward
    the combined serial rate. Keep GpSimd to at most one light 1-stream TS
    (the clamp); its TT/CAST ops are slow (4.6us/7.3us per [128,2048]) and
    poison DVE. All other elementwise work belongs on DVE at clean rates.
  - DVE TT with a [0,16]-broadcast operand runs 1 elem/cycle regardless of
    dtype; duplicating the broadcast values ("pairs", inner AP [0,8][1,2])
    restores 2 elem/cycle for 16-bit. TS (1-stream) ops hit ~3 elem/cycle
    16-bit, ~1.7 f32. f32 streams cap at 1 elem/cycle. f32-out TTs from
    16-bit inputs run half rate. tensor_reduce is 1 elem/cycle regardless.
  - int8 is no faster: CAST->int8 1228ns, TT int8 in0 2302ns ([128,2048]),
    though CAST bf16->int8 is exactly clip(RNE(v),-128,127) if ever useful.
  - PE matmuls pipeline to ~215ns per [128c,512f] bf16 when rhs is a
    contiguous tile; strided rhs APs halve throughput. DoubleRow perf mode
    (2x) is fp8-only on TRN2.
  - dma_start_transpose (xbar, 2-byte dtypes, out[do,di,m]=in[m,di*128+do])
    works but costs ~2x normal DMA engine-time per byte; transposing every
    chunk saturated all 16 DMA engines (~127us each). PE transpose + ACT
    PSUM->SBUF copy is cheaper when ACT has headroom.
  - DMA cannot read PSUM (API assert); GpSimd TS from PSUM fails NEFF
    codegen; ACT reads PSUM at ~1 elem/cycle. Matmul operands must be SBUF.
  - TS cannot mix bitwise and arithmetic ops in one instruction; TS on Pool
    only does min/max; no negative int immediates on TS; scalar slots of
    activation() take [P,1] APs or immediates - exact-pow2 scale immediates
    ride ACT copies for free (used for the 0.5 output fix-up).
  - Upfront DMA bursts round-robin packets across engines: a tensor needed
    first should be dispatched first (x0 before the other prefetches).
"""

import os
import sys

import numpy as np

for _p in ("/opt/trn_rl_repo",):
    if _p not in sys.path and os.path.isdir(_p):
        sys.path.append(_p)

N_CORES = 8

# DVE and GpSimd share one SBUF port (concurrent ops degrade to the
# combined serial rate), so all elementwise work runs on DVE at clean rates
# and GpSimd stays idle. ACT (own port) does convert/pairs/outcopy.
CFG = {
    "vdt": "bf16",     # v/u/t dtype: bf16 (fast, double-round) | f32 (exact)
    "mult": "v",
    "clamp": "g",
    "round": "v",
    "scale_dve_groups": 128,  # of 128 groups/chunk: DVE share of scale TT
    "outcopy": "a",
}

_CACHE = {}


def _eng(nc, which, idx=0):
    s = {"v": nc.vector, "g": nc.gpsimd, "a": nc.scalar}
    return s[which[idx % len(which)]]


def _bcast_group_ap(t, G, sz):
    """AP reading tile t[P, G] as [P, G, sz] with the last dim broadcast."""
    import concourse.bass as bass

    ap = t.ap.copy()
    ap.append([0, sz])
    return bass.AP(tensor=t.tensor, offset=t.offset, ap=ap)


def _pairs_ap(t, g0, ng):
    """Read pairs-packed tile t[P, 2G] (each group value duplicated) as
    [P, ng, 8, 2] covering groups [g0, g0+ng): coalesced 4B pair reads."""
    import concourse.bass as bass

    view = t[:, 2 * g0 : 2 * (g0 + ng)]
    ap = view.ap.copy()[:1]
    ap.append([2, ng])
    ap.append([0, 8])
    ap.append([1, 2])
    return bass.AP(tensor=view.tensor, offset=view.offset, ap=ap)


def _build(nrows, K, O, x_bit, w_bit, x_sz, w_sz, cfg=None):
    import concourse.bacc as bacc
    import concourse.bass as bass  # noqa: F401
    import concourse.mybir as mybir
    import concourse.tile as tile
    from concourse.masks import make_identity

    cfg = dict(CFG, **(cfg or {}))
    f32 = mybir.dt.float32
    bf16 = mybir.dt.bfloat16
    i32 = mybir.dt.int32
    A = mybir.AluOpType

    assert x_bit == w_bit == 8 and x_sz == w_sz == 16
    bit, sz = x_bit, x_sz
    EM = 0x7F800000
    KADD = (bit - 2) << 23
    # v = x*2^(1-e) lives on the 2^-(bit-2) grid: C and the clamp bounds are
    # scaled by 2^-(bit-2); xq = t*2^e comes out 2x the reference, fixed by a
    # 0.5 scale immediate in the outcopy (all exact powers of two).
    C = float(np.float32(1.5 * 2.0 ** (23 - (bit - 2))))
    qhi = float((2 ** (bit - 1) - 1) * 2.0 ** (-(bit - 2)))
    qlo = float(-(2 ** (bit - 1)) * 2.0 ** (-(bit - 2)))

    P = 128
    RPC = 512
    assert nrows % RPC == 0
    n_chunks = nrows // RPC
    FB = RPC // P            # 4 row-blocks per chunk
    F = FB * K               # 2048 free columns per chunk
    G = F // sz              # 128 groups per chunk
    KB = K // P              # 4 k-blocks
    OB = O // P              # 4 o-blocks
    GW = K // sz             # 32 groups per weight row-tile
    f16 = mybir.dt.float16
    # fp16 intermediates: 11-bit mantissa cuts the pre-round double-rounding
    # error ~8x vs bf16 (maxerr 0.021 vs 0.035) at identical 16-bit DVE rates
    vdt = f16 if cfg["vdt"] == "bf16" else f32
    sdg = cfg["scale_dve_groups"]

    nc = bacc.Bacc("TRN2", debug=False)
    x_d = nc.dram_tensor("x", (nrows, K), f32, kind="ExternalInput").ap()
    wt_d = nc.dram_tensor("wqt", (K, O), bf16, kind="ExternalInput").ap()
    o_d = nc.dram_tensor("out", (nrows, O), bf16, kind="ExternalOutput").ap()

    with tile.TileContext(nc) as tc:
        with (
            tc.tile_pool(name="const", bufs=1) as constp,
            tc.tile_pool(name="xraw", bufs=4) as xraw,
            tc.tile_pool(name="sml", bufs=4) as sml,
            tc.tile_pool(name="xb", bufs=4) as xbp,
            tc.tile_pool(name="prs", bufs=4) as prs,
            tc.tile_pool(name="v", bufs=3) as vp,
            tc.tile_pool(name="u", bufs=2) as up,
            tc.tile_pool(name="t", bufs=2) as tp,
            tc.tile_pool(name="xq", bufs=4) as xqp,
            tc.tile_pool(name="xqT", bufs=4) as xqTp,
            tc.tile_pool(name="osb", bufs=4) as osb,
            tc.tile_pool(name="psO", bufs=2, space="PSUM") as psO,
            tc.tile_pool(name="psT", bufs=2, space="PSUM") as psT,
        ):
            p2s = constp.tile([P, 1], f32)
            nc.vector.memset(p2s, float(2.0 ** (-(bit - 1))))
            r2s = constp.tile([P, 1], f32)
            nc.vector.memset(r2s, float(2.0 ** (bit - 2)))
            ident = constp.tile([P, P], bf16)
            make_identity(nc, ident)

            # ---- software-pipelined main loop over items (r0, nf) ----
            # First and last chunks are split into halves: a shorter first
            # chain fills the pipeline sooner, a shorter last chain halves
            # the serial drain tail.
            items = (
                [(0, FB // 2), (RPC // 2, FB // 2)]
                + [(c * RPC, FB) for c in range(1, n_chunks - 1)]
                + [((n_chunks - 1) * RPC, FB // 2),
                   ((n_chunks - 1) * RPC + RPC // 2, FB // 2)]
            )
            n_items = len(items)
            st = {}

            def dma_in(i):
                r0, nf = items[i]
                x_raw = xraw.tile([P, FB, K], f32, tag="x_raw", name="x_raw")[:, :nf]
                src = x_d[r0 : r0 + nf * P, :].rearrange("(f p) k -> p f k", p=P)
                nc.sync.dma_start(out=x_raw, in_=src)
                st[i] = {"x": x_raw}

            def quant_a(i):
                r0, nf = items[i]
                Fi = nf * K
                Gi = Fi // sz
                s = st[i]
                xt = s["x"].rearrange("p f k -> p (f k)")
                s["xt"] = xt
                gmx = sml.tile([P, F // sz], f32, tag="gmx", name="gmx")[:, :Gi]
                nc.vector.tensor_reduce(
                    out=gmx,
                    in_=xt.rearrange("p (g s) -> p g s", s=sz),
                    axis=mybir.AxisListType.X,
                    op=A.max,
                    apply_absolute_value=True,
                )
                sc2 = sml.tile([P, 2, Gi], i32, tag=f"sc2{nf}", name="sc2")
                nc.vector.tensor_scalar(
                    out=sc2[:, 0], in0=gmx.bitcast(i32), scalar1=EM, scalar2=EM,
                    op0=A.bitwise_and, op1=A.bitwise_xor,
                )
                nc.vector.tensor_scalar(
                    out=sc2[:, 1], in0=sc2[:, 0], scalar1=EM, scalar2=None,
                    op0=A.bitwise_xor,
                )
                # x -> bf16 on ACT (deep-skewed off the critical chain);
                # the mult then runs 1460ns (bf16 pairs) vs 2291ns (f32 in0)
                xb = xbp.tile([P, F], f16, tag="xb", name="xb")[:, :Fi]
                nc.scalar.copy(xb, xt)
                s["xb"] = xb
                prt = prs.tile([P, 2, 2 * Gi], f16, tag=f"prt{nf}", name="prt")
                nc.vector.tensor_copy(
                    out=prt.rearrange("p a b -> p (a b)"),
                    in_=_bcast_group_ap(
                        sc2.rearrange("p a b -> p (a b)").bitcast(f32), 2 * Gi, 2
                    ),
                )
                s["rbp"], s["scp"] = prt[:, 0], prt[:, 1]

            def quant_b(i):
                r0, nf = items[i]
                Fi = nf * K
                Gi = Fi // sz
                s = st.pop(i)
                v = vp.tile([P, F], vdt, tag="v", name="v")[:, :Fi]
                _eng(nc, cfg["mult"], i).tensor_tensor(
                    out=v, in0=s["xb"], in1=_pairs_ap(s["rbp"], 0, Gi), op=A.mult,
                )
                u = up.tile([P, F], vdt, tag="u", name="u")[:, :Fi]
                _eng(nc, cfg["clamp"], i).tensor_scalar(
                    out=u, in0=v, scalar1=qhi + 0.0, scalar2=qlo + 0.0,
                    op0=A.min, op1=A.max,
                )
                t = tp.tile([P, F], f16, tag="t", name="t")[:, :Fi]
                _eng(nc, cfg["round"], i).tensor_scalar(
                    out=t, in0=u, scalar1=C, scalar2=-C, op0=A.add, op1=A.add,
                )
                xq = xqp.tile([P, F], bf16, tag="xq", name="xq")[:, :Fi]
                sd = (sdg * nf) // FB
                if sd > 0:
                    nc.vector.tensor_tensor(
                        out=xq[:, : sd * sz], in0=t[:, : sd * sz],
                        in1=_pairs_ap(s["scp"], 0, sd), op=A.mult,
                    )
                if sd < Gi:
                    nc.gpsimd.tensor_tensor(
                        out=xq[:, sd * sz :], in0=t[:, sd * sz :],
                        in1=_pairs_ap(s["scp"], sd, Gi - sd), op=A.mult,
                    )
                xq_nat = xq.rearrange("p (f c q) -> p f c q", f=nf, c=KB)
                ptT_full = psT.tile([P, FB * KB, P], bf16, tag="ptT")
                ptT = ptT_full[:, : nf * KB, :]
                for fb in range(nf):
                    for kb in range(KB):
                        nc.tensor.transpose(
                            ptT[:, fb * KB + kb, :], xq_nat[:, fb, kb], ident
                        )
                xqT = xqTp.tile([P, FB * KB, P], bf16, tag="xqT", name="xqT")[:, : nf * KB]
                nc.scalar.copy(xqT, ptT)
                st[i] = {"xqT": xqT}

            def mm_out(i):
                r0, nf = items[i]
                s = st.pop(i)
                xqT = s["xqT"]
                for fp in range(nf // 2):
                    po = psO.tile([P, 2, O], f32, tag="po")
                    for g in range(2):
                        fb = fp * 2 + g
                        for kb in range(KB):
                            nc.tensor.matmul(
                                po[:, g, :],
                                lhsT=xqT[:, fb * KB + kb, :],
                                rhs=wqT[kb],
                                start=(kb == 0),
                                stop=(kb == KB - 1),
                            )
                    out_sb = osb.tile([P, 2, O], bf16, tag="out_sb")
                    nc.scalar.activation(
                        out_sb, po, mybir.ActivationFunctionType.Copy, scale=0.5
                    )
                    rr = r0 + fp * 2 * P
                    dst = o_d[rr : rr + 2 * P, :].rearrange("(f p) k -> p f k", p=P)
                    nc.sync.dma_start(out=dst, in_=out_sb)

            dma_in(0)
            # weights after x0: x0's input is on the critical path, the wqT
            # tiles aren't needed until the first matmul (~25us in)
            wqT = []
            for kb in range(KB):
                wt = constp.tile([P, O], bf16, tag=f"wqT{kb}", bufs=KB)
                nc.sync.dma_start(out=wt, in_=wt_d[kb * P : (kb + 1) * P, :])
                wqT.append(wt)
            for j in range(1, min(4, n_items)):
                dma_in(j)
            quant_a(0)
            if n_items > 1:
                quant_a(1)
            mm_done = set()

            def mm(j):
                if j not in mm_done:
                    mm_done.add(j)
                    mm_out(j)

            for i in range(n_items):
                if i + 4 < n_items:
                    dma_in(i + 4)
                quant_b(i)
                if i + 2 < n_items:
                    quant_a(i + 2)
                if i >= 1:
                    mm(i - 1)
                if i >= n_items - 2:
                    mm(i)
    nc.compile()
    return nc


def _get_program(nrows, K, O, x_bit, w_bit, x_sz, w_sz):
    key = (nrows, K, O, x_bit, w_bit, x_sz, w_sz)
    if key not in _CACHE:
        _CACHE[key] = _build(nrows, K, O, x_bit, w_bit, x_sz, w_sz)
    return _CACHE[key]


def _host_bfp_quantize(w, bit, sz):
    """Reference bfp_quantize in float32 numpy (np.round is RNE like jnp)."""
    g = w.reshape(-1, sz)
    maxabs = np.max(np.abs(g), axis=1, keepdims=True)
    e = np.floor(np.log2(np.maximum(maxabs, np.float32(1e-38), dtype=np.float32)))
    scale = np.exp2(e - (bit - 1), dtype=np.float32)
    qmax = np.float32(2.0 ** (bit - 1) - 1.0)
    q = np.clip(np.round(g / scale), -qmax - 1.0, qmax) * scale
    return q.reshape(w.shape).astype(np.float32)


def kernel(input, weight, bias, i_bit, i_sz, w_bit, w_sz):
    import ml_dtypes
    from concourse.bass_utils import run_bass_kernel_spmd

    x = np.ascontiguousarray(np.asarray(input, dtype=np.float32))
    w = np.ascontiguousarray(np.asarray(weight, dtype=np.float32))
    b = np.asarray(bias, dtype=np.float32).reshape(1, -1)
    i_bit, i_sz, w_bit, w_sz = int(i_bit), int(i_sz), int(w_bit), int(w_sz)

    N, K = x.shape
    O = w.shape[0]
    assert N % N_CORES == 0
    shard = N // N_CORES

    # wq values are int*2^e, exactly representable in bf16
    wqt = np.ascontiguousarray(
        _host_bfp_quantize(w, w_bit, w_sz).T
    ).astype(ml_dtypes.bfloat16)

    nc = _get_program(shard, K, O, i_bit, w_bit, i_sz, w_sz)
    in_maps = [
        {"x": x[i * shard : (i + 1) * shard], "wqt": wqt} for i in range(N_CORES)
    ]
    res = run_bass_kernel_spmd(nc, in_maps, list(range(N_CORES)))
    out = np.empty((N, O), dtype=np.float32)
    for i, r in enumerate(res.results):
        np.add(
            np.asarray(r["out"]).astype(np.float32), b,
            out=out[i * shard : (i + 1) * shard],
        )
    return out
